# revision 25
# baseline (speedup 1.0000x reference)
"""CrossAttention Trainium2 kernel, 8-core SPMD, single-pass fp16.

Sharding: core c -> (batch b = c//2, head-group g = c%2).  Each core computes
8 of the 16 heads for one batch: q/k/v projections restricted to its
inner-dim slice [g*512:(g+1)*512], full attention for those heads, and a
partial output projection (contraction over its 512 inner dims).  Host
pre-transposes x/context, casts everything to fp16, and sums the two
partial outputs per batch + bias.

All matmuls run in fp16 (1 cycle/row on the PE like bf16, but 8x finer
mantissa: rel err ~4e-4 vs tolerance 2e-2).  All intermediates (Q^T, K^T,
V, attention output) stay resident in SBUF.

Schedule: attention is split into two n-halves.  Within a half, sim+exp
for head h+1 is software-pipelined into attn@v of head h so ACT overlaps
PE.  Projection work is dribbled into the attention windows so the PE
never idles while ACT catches up on exp: half0 absorbs vproj and most of
qproj; half1 absorbs the last qproj chunks plus oproj for half0's rows.
attn@v accumulators are normalized (approx reciprocal + partition
broadcast + multiply) off the PE critical path, written by DVE directly
into the aot tiles (no SBUF-SBUF DMA).

The pre-attention region is input-DMA-bandwidth-bound (~25us for 8.9MB);
kvproj + the first qproj chain hide inside it.  Inputs stream on two DMA
queues (Sync: per-k ch/wk; Scalar: wq/xh/wv/wo as large strided
transfers).  Output is fp16 (error contribution ~5e-5) written per
n-tile; tail oproj copies alternate Scalar/Vector engines.

PSUM budget (8 banks): sim pool 2x[128,1024] (4 banks) + po pool
3x[65,512] (3 banks) + dribble pool 1x[128,512] (1 bank).

Per-core dataflow:
  qt[i]  = Wq_s^T @ x^T        [512, 2048]  (scale folded into Wq)
  kt[i]  = Wk^T @ c^T          [512, 1024]
  va[mi] = c @ Wv augmented    [128, 8*65]  col h*65+64 == 1.0 so the
           softmax denominator rides the attn@v matmul
  per head h: simT = kt_h slices @ qt_h (K=64) -> exp -> es fp16
              po[65,512] = va_h^T @ es     row 64 = denominator
              aot slice = po[0:64]*approx(1/po[64])
  out[n,1024] = aot^T-slices @ Wo_s    (partial; host adds pair + bias)
"""
import sys

sys.path.insert(0, "/opt/trn_rl_repo")

import numpy as np

import concourse.bacc as bacc
import concourse.mybir as mybir
import concourse.tile as tile
from concourse.bass_utils import run_bass_kernel_spmd

# bass_utils imports antenv.axon_hooks when trace=True; the read-only antenv
# package in this image lacks it, so register a no-op stub if missing.
try:
    import antenv.axon_hooks  # noqa: F401
except ImportError:
    import types as _types

    _stub = _types.ModuleType("antenv.axon_hooks")
    _stub.get_axon_ntff_profile_hook = lambda: None
    _stub.set_axon_ntff_profile_hook = lambda h: None
    sys.modules["antenv.axon_hooks"] = _stub

F32 = mybir.dt.float32
F16 = mybir.dt.float16
EXP = mybir.ActivationFunctionType.Exp

B, N, M = 4, 2048, 1024
QD, CD = 1024, 768
HEADS, DH = 16, 64
INNER = HEADS * DH
HG = 8            # heads per core
IS = HG * DH      # inner slice per core = 512
NC = 8

KQ = QD // 128    # 8
KC = CD // 128    # 6
NJ = N // 512     # 4
MT = M // 128     # 8
IT = IS // 128    # 4
NT = N // 128     # 16

LAST_RESULTS = None  # stashed BassKernelResults for test.py introspection


def build_nc():
    nc = bacc.Bacc("TRN2", target_bir_lowering=False, debug=False, num_devices=NC)

    def din(name, shape):
        return nc.dram_tensor(name, shape, F16, kind="ExternalInput").ap()

    xT = din("xT", [QD, N])
    cT = din("cT", [CD, M])
    wq = din("wq", [QD, IS])
    wk = din("wk", [CD, IS])
    wv = din("wv", [CD, IS])
    wo = din("wo", [IS, QD])
    out = nc.dram_tensor("out", [N, QD], F16, kind="ExternalOutput").ap()

    with tile.TileContext(nc) as tc:
        with tc.tile_pool(name="res", bufs=1) as resp, \
             tc.tile_pool(name="es", bufs=1) as esp, \
             tc.tile_pool(name="nm", bufs=3) as nmp, \
             tc.tile_pool(name="ob", bufs=3) as obp, \
             tc.tile_pool(name="qp", bufs=1) as qpp, \
             tc.tile_pool(name="kvp", bufs=1) as kvp, \
             tc.tile_pool(name="ps", bufs=2, space="PSUM") as psp, \
             tc.tile_pool(name="po", bufs=3, space="PSUM") as pop, \
             tc.tile_pool(name="dr", bufs=1, space="PSUM") as drp:

            # Few large tiles (not per-index lists): every (tag, buf) pair
            # owns a semaphore that the end-of-program barrier waits on per
            # engine queue at ~115ns each — merging tiles shrinks the
            # ~12us teardown tail.
            qt = resp.tile([128, IT * N], F16, tag="qt", name="qt")
            # Per-head zero-padded K^T: rows r:r+64 hold head h's K, the
            # other 64 rows are zero.  This makes every matmul in the
            # attention stream a 128-row-stationary op — PE tile-geometry
            # (row-size) switches between 64- and 128-row stationaries cost
            # ~105ns each, ~250 times per kernel.
            kz = resp.tile([128, HG * M], F16, tag="kz", name="kz")
            va = resp.tile([128, MT * HG * 65], F16, tag="va", name="va")
            # aot split by n-half so half1 oproj never (falsely) waits on
            # half1 normalize writes.
            aoh = [resp.tile([128, IT * 1024], F16, tag=f"ao{j}", name=f"ao{j}")
                   for j in range(2)]
            wo_sb = resp.tile([128, IT, QD], F16, tag="wo", name="wo")
            wq_sb = qpp.tile([128, KQ, IS], F16, tag="wq", name="wq")
            xh = qpp.tile([128, KQ, N], F16, tag="xh", name="xh")
            ch = kvp.tile([128, KC, M], F16, tag="ch", name="ch")
            wk_sb = kvp.tile([128, KC, IS], F16, tag="wk", name="wk")
            wv_sb = kvp.tile([128, KC, IS], F16, tag="wv", name="wv")

            # Zero the pad halves of kz on GpSimd (idle during input DMA).
            for h in range(HG):
                r = (h % 2) * 64
                nc.gpsimd.memset(kz[64 - r:128 - r, h * M:(h + 1) * M], 0.0)

            def blk(src, k0, k1):
                return src[k0 * 128:k1 * 128, :].rearrange(
                    "(k p) f -> p k f", p=128)

            # Input DMAs on two hardware queues (aggregate HBM bandwidth is
            # the binding constraint at ~25us for 8.9MB; the split just
            # lets kvproj start early while xh streams).  Sync: kvproj's
            # tensors per-k + wv; Scalar queue: qproj/oproj tensors as a
            # few large strided transfers.
            for k in range(KC):
                ksl = slice(k * 128, (k + 1) * 128)
                nc.sync.dma_start(wk_sb[:, k, :], wk[ksl, :])
                nc.sync.dma_start(ch[:, k, :], cT[ksl, :])
            nc.scalar.dma_start(wq_sb[:], blk(wq, 0, 8))
            nc.scalar.dma_start(xh[:, 0:4, :], blk(xT, 0, 4))
            nc.scalar.dma_start(wv_sb[:], blk(wv, 0, 6))
            nc.scalar.dma_start(xh[:, 4:8, :], blk(xT, 4, 8))
            nc.scalar.dma_start(wo_sb[:], blk(wo, 0, 4))

            def ps2():
                return psp.tile([128, 1024], F32, tag="mm", name="mm")

            # ------------- K^T -> kz (per-head, zero-padded) -------------
            with nc.named_scope("kvproj"):
                for i in range(IT):
                    isl = slice(i * 128, (i + 1) * 128)
                    pk = ps2()
                    for k in range(KC):
                        for jm in range(2):
                            nc.tensor.matmul(pk[:, jm * 512:(jm + 1) * 512],
                                             wk_sb[:, k, isl],
                                             ch[:, k, jm * 512:(jm + 1) * 512],
                                             start=(k == 0), stop=(k == KC - 1))
                    nc.vector.tensor_copy(
                        kz[0:64, (2 * i) * M:(2 * i + 1) * M], pk[0:64, :])
                    nc.vector.tensor_copy(
                        kz[64:128, (2 * i + 1) * M:(2 * i + 2) * M],
                        pk[64:128, :])

            def vproj_chunk(mi):
                # V -> va for one m-chunk, on the 1-bank dribble pool
                pd = drp.tile([128, 512], F32, tag="dr", name="dr")
                msl = slice(mi * 128, (mi + 1) * 128)
                for k in range(KC):
                    nc.tensor.matmul(pd[:], ch[:, k, msl], wv_sb[:, k, :],
                                     start=(k == 0), stop=(k == KC - 1))
                hcol = va[:, mi * 520:(mi + 1) * 520].rearrange(
                    "p (h c) -> p h c", c=65)
                psv = pd[:].rearrange("p (h c) -> p h c", c=64)
                nc.vector.memset(hcol[:, :, 64], 1.0)
                nc.vector.tensor_copy(hcol[:, :, 0:64], psv[:])

            # ------------- Q^T -> qt -------------
            def qproj_mm(i, jn, ps, psl):
                isl = slice(i * 128, (i + 1) * 128)
                for k in range(KQ):
                    nc.tensor.matmul(ps[:, psl], wq_sb[:, k, isl],
                                     xh[:, k, jn * 512:(jn + 1) * 512],
                                     start=(k == 0), stop=(k == KQ - 1))

            # Only head-pair 0's q is projected up front (it gates the
            # first sim); the rest dribbles into the attention windows.
            with nc.named_scope("qproj0"):
                for i in range(1):
                    pq = ps2()
                    for jn in range(2):
                        qproj_mm(i, jn, pq, slice(jn * 512, (jn + 1) * 512))
                    nc.vector.tensor_copy(qt[:, i * N:i * N + 1024], pq[:])

            # dribble generators: emitted one chunk per attention unit to
            # fill PE while ACT works through exp.  Order matters (PE
            # executes in emission order): qt[2]/qt[3] first-half chunks
            # must land before sim of heads 4..7 is emitted.
            def qdrib(chunks):
                for i, jn in chunks:
                    pd = drp.tile([128, 512], F32, tag="dr", name="dr")
                    qproj_mm(i, jn, pd, slice(0, 512))
                    nc.vector.tensor_copy(
                        qt[:, i * N + jn * 512:i * N + (jn + 1) * 512], pd[:])
                    yield

            def oproj_half(nts):
                for nt in nts:
                    tsl = slice((nt % 8) * 128, (nt % 8 + 1) * 128)
                    ao = aoh[nt // 8]
                    ob = obp.tile([128, QD], F16, tag="ob", name="ob")
                    for half in range(2):
                        qsl = slice(half * 512, (half + 1) * 512)
                        pd = drp.tile([128, 512], F32, tag="dr", name="dr")
                        for ik in range(IT):
                            nc.tensor.matmul(
                                pd[:],
                                ao[:, ik * 1024 + tsl.start:ik * 1024 + tsl.stop],
                                wo_sb[:, ik, qsl],
                                start=(ik == 0), stop=(ik == IT - 1))
                        nc.vector.tensor_copy(ob[:, qsl], pd[:])
                        yield
                    nc.sync.dma_start(out[nt * 128:(nt + 1) * 128, :], ob[:])

            def chain(*gens):
                for g in gens:
                    yield from g

            # ---------------- attention ----------------
            with nc.named_scope("attn"):
                for jnp in range(2):
                    if jnp == 0:
                        dribble = qdrib([(1, 0), (1, 1), (2, 0), (2, 1),
                                         (3, 0), (3, 1), (0, 2), (1, 2),
                                         (2, 2), (0, 3), (1, 3)])
                    else:
                        dribble = chain(qdrib([(2, 3), (3, 2), (3, 3)]),
                                        oproj_half(range(8)))
                    es_cur = {}

                    def sim_block(h, mi, jnp=jnp, es_cur=None):
                        i = h // 2
                        ktsl = kz[:, h * M + mi * 128:h * M + (mi + 1) * 128]
                        pss = ps2()
                        for sub in range(2):
                            jn = jnp * 2 + sub
                            nc.tensor.matmul(
                                pss[:, sub * 512:(sub + 1) * 512], ktsl,
                                qt[:, i * N + jn * 512:i * N + (jn + 1) * 512],
                                start=True, stop=True)
                        es = esp.tile([128, 1024], F16, tag=f"es{mi}",
                                      name=f"es{mi}", bufs=2)
                        nc.scalar.activation(es[:], pss[:], EXP)
                        es_cur[mi] = es

                    for mi in range(MT):
                        sim_block(0, mi, es_cur=es_cur)
                        if jnp == 0:
                            # va[mi] must be emitted before attn@v of head 0
                            vproj_chunk(mi)
                            # qt[1] chunks must land before sim of heads 2/3
                            if mi % 4 == 3:
                                next(dribble, None)
                        elif mi % 2 == 1:
                            next(dribble, None)
                    for h in range(HG):
                        po = [pop.tile([65, 512], F32, tag="po", name="po")
                              for _ in range(2)]
                        es_prev, es_cur = es_cur, {}
                        for mi in range(MT):
                            for sub in range(2):
                                nc.tensor.matmul(
                                    po[sub][:],
                                    va[:, mi * 520 + h * 65:
                                         mi * 520 + h * 65 + 65],
                                    es_prev[mi][:, sub * 512:(sub + 1) * 512],
                                    start=(mi == 0), stop=(mi == MT - 1))
                            if h + 1 < HG:
                                sim_block(h + 1, mi, es_cur=es_cur)
                            elif mi % 2 == 0:
                                next(dribble, None)
                            if mi % 4 == 3:
                                next(dribble, None)
                        i, r = h // 2, (h % 2) * 64
                        for sub in range(2):
                            # denominator row -> partition 0 first: the
                            # approx reciprocal (custom DVE op) requires
                            # SBUF input at partition offset 0.
                            drow = nmp.tile([1, 512], F32, tag="drow",
                                            name="drow", bufs=2)
                            nc.vector.tensor_copy(drow[:], po[sub][64:65, :])
                            rf = nmp.tile([1, 512], F32, tag="rf", name="rf",
                                          bufs=2)
                            nc.vector.reciprocal_approx_fast(rf[:], drow[:])
                            pbs = nmp.tile([64, 512], F32, tag="pbs",
                                           name="pbs", bufs=2)
                            nc.gpsimd.partition_broadcast(pbs[:], rf[:])
                            nsl = slice(i * 1024 + sub * 512,
                                        i * 1024 + (sub + 1) * 512)
                            nc.vector.tensor_mul(aoh[jnp][r:r + 64, nsl],
                                                 po[sub][0:64, :], pbs[:])
                    # drain any dribbles not consumed during this half
                    for _ in dribble:
                        pass

            # -------- out tail: second n-half of oproj --------
            # uses the (now free) 2-buf mm pool for pipelining; ob copies
            # go through the Scalar engine, idle once attention is done.
            with nc.named_scope("oproj"):
                for nt in range(8, NT):
                    t0 = (nt - 8) * 128
                    pso = ps2()
                    for ik in range(IT):
                        for half in range(2):
                            qsl = slice(half * 512, (half + 1) * 512)
                            nc.tensor.matmul(
                                pso[:, qsl],
                                aoh[1][:, ik * 1024 + t0:ik * 1024 + t0 + 128],
                                wo_sb[:, ik, qsl],
                                start=(ik == 0), stop=(ik == IT - 1))
                    ob = obp.tile([128, QD], F16, tag="ob", name="ob")
                    # alternate the PSUM->SBUF copies between Scalar and
                    # Vector so neither engine paces the 2-buf pso rotation
                    if nt % 2 == 0:
                        nc.scalar.copy(ob[:], pso[:])
                    else:
                        nc.vector.tensor_copy(ob[:], pso[:])
                    nc.sync.dma_start(out[nt * 128:(nt + 1) * 128, :], ob[:])
    nc.compile()
    return nc


_NC_CACHE = None


def kernel(x, context, Wq, Wk, Wv, Wo, bo, _trace=False):
    global _NC_CACHE, LAST_RESULTS
    f16 = np.float16

    x = np.asarray(x, np.float32)
    context = np.asarray(context, np.float32)
    Wq = np.asarray(Wq, np.float32)
    Wk = np.asarray(Wk, np.float32)
    Wv = np.asarray(Wv, np.float32)
    Wo = np.asarray(Wo, np.float32)
    scale = np.float32(DH ** -0.5)

    if _NC_CACHE is None:
        _NC_CACHE = build_nc()
    nc = _NC_CACHE

    in_maps = []
    for c in range(NC):
        b, g = c // 2, c % 2
        sl = slice(g * IS, (g + 1) * IS)
        in_maps.append({
            "xT": np.ascontiguousarray(x[b].T).astype(f16),
            "cT": np.ascontiguousarray(context[b].T).astype(f16),
            "wq": (Wq[:, sl] * scale).astype(f16),
            "wk": np.ascontiguousarray(Wk[:, sl]).astype(f16),
            "wv": np.ascontiguousarray(Wv[:, sl]).astype(f16),
            "wo": np.ascontiguousarray(Wo[sl, :]).astype(f16),
        })
    res = run_bass_kernel_spmd(nc, in_maps, core_ids=list(range(NC)),
                               trace=_trace)
    LAST_RESULTS = res
    out = np.empty((B, N, QD), np.float32)
    bo32 = np.asarray(bo, np.float32)
    for b in range(B):
        out[b] = (res.results[2 * b]["out"].astype(np.float32)
                  + res.results[2 * b + 1]["out"].astype(np.float32) + bo32)
    return out


# revision 28
# speedup vs baseline: 1.0155x; 1.0155x over previous
"""CrossAttention Trainium2 kernel, 8-core SPMD, single-pass fp16.

Sharding: core c -> (batch b = c//2, head-group g = c%2).  Each core computes
8 of the 16 heads for one batch: q/k/v projections restricted to its
inner-dim slice [g*512:(g+1)*512], full attention for those heads, and a
partial output projection (contraction over its 512 inner dims).  Host
pre-transposes x/context, casts everything to fp16, and sums the two
partial outputs per batch + bias.

All matmuls run in fp16 (1 cycle/row on the PE like bf16, but 8x finer
mantissa: rel err ~4e-4 vs tolerance 2e-2).  All intermediates (Q^T, K^T,
V, attention output) stay resident in SBUF.

Schedule: attention is split into two n-halves.  Within a half, sim+exp
for head h+1 is software-pipelined into attn@v of head h so ACT overlaps
PE.  Projection work is dribbled into the attention windows so the PE
never idles while ACT catches up on exp: half0 absorbs vproj and most of
qproj; half1 absorbs the last qproj chunks plus oproj for half0's rows.
attn@v accumulators are normalized (approx reciprocal + partition
broadcast + multiply) off the PE critical path, written by DVE directly
into the aot tiles (no SBUF-SBUF DMA).

The pre-attention region is input-DMA-bandwidth-bound (~25us for 8.9MB);
kvproj + the first qproj chain hide inside it.  Inputs stream on two DMA
queues (Sync: per-k ch/wk; Scalar: wq/xh/wv/wo as large strided
transfers).  Output is fp16 (error contribution ~5e-5) written per
n-tile; tail oproj copies alternate Scalar/Vector engines.

PSUM budget (8 banks): sim pool 2x[128,1024] (4 banks) + po pool
3x[65,512] (3 banks) + dribble pool 1x[128,512] (1 bank).

Per-core dataflow:
  qt[i]  = Wq_s^T @ x^T        [512, 2048]  (scale folded into Wq)
  kt[i]  = Wk^T @ c^T          [512, 1024]
  va[mi] = c @ Wv augmented    [128, 8*65]  col h*65+64 == 1.0 so the
           softmax denominator rides the attn@v matmul
  per head h: simT = kt_h slices @ qt_h (K=64) -> exp -> es fp16
              po[65,512] = va_h^T @ es     row 64 = denominator
              aot slice = po[0:64]*approx(1/po[64])
  out[n,1024] = aot^T-slices @ Wo_s    (partial; host adds pair + bias)
"""
import sys

sys.path.insert(0, "/opt/trn_rl_repo")

import numpy as np

import concourse.bacc as bacc
import concourse.mybir as mybir
import concourse.tile as tile
from concourse.bass_utils import run_bass_kernel_spmd

# bass_utils imports antenv.axon_hooks when trace=True; the read-only antenv
# package in this image lacks it, so register a no-op stub if missing.
try:
    import antenv.axon_hooks  # noqa: F401
except ImportError:
    import types as _types

    _stub = _types.ModuleType("antenv.axon_hooks")
    _stub.get_axon_ntff_profile_hook = lambda: None
    _stub.set_axon_ntff_profile_hook = lambda h: None
    sys.modules["antenv.axon_hooks"] = _stub

F32 = mybir.dt.float32
F16 = mybir.dt.float16
EXP = mybir.ActivationFunctionType.Exp

B, N, M = 4, 2048, 1024
QD, CD = 1024, 768
HEADS, DH = 16, 64
INNER = HEADS * DH
HG = 8            # heads per core
IS = HG * DH      # inner slice per core = 512
NC = 8

KQ = QD // 128    # 8
KC = CD // 128    # 6
NJ = N // 512     # 4
MT = M // 128     # 8
IT = IS // 128    # 4
NT = N // 128     # 16

LAST_RESULTS = None  # stashed BassKernelResults for test.py introspection


def build_nc():
    nc = bacc.Bacc("TRN2", target_bir_lowering=False, debug=False, num_devices=NC)

    def din(name, shape):
        return nc.dram_tensor(name, shape, F16, kind="ExternalInput").ap()

    xT = din("xT", [QD, N])
    cT = din("cT", [CD, M])
    wq = din("wq", [QD, IS])
    wk = din("wk", [CD, IS])
    wv = din("wv", [CD, IS])
    wo = din("wo", [IS, QD])
    out = nc.dram_tensor("out", [N, QD], F16, kind="ExternalOutput").ap()

    with tile.TileContext(nc) as tc:
        with tc.tile_pool(name="res", bufs=1) as resp, \
             tc.tile_pool(name="es", bufs=1) as esp, \
             tc.tile_pool(name="nm", bufs=3) as nmp, \
             tc.tile_pool(name="ob", bufs=3) as obp, \
             tc.tile_pool(name="qp", bufs=1) as qpp, \
             tc.tile_pool(name="kvp", bufs=1) as kvp, \
             tc.tile_pool(name="ps", bufs=2, space="PSUM") as psp, \
             tc.tile_pool(name="po", bufs=3, space="PSUM") as pop, \
             tc.tile_pool(name="dr", bufs=1, space="PSUM") as drp:

            # Few large tiles (not per-index lists): every (tag, buf) pair
            # owns a semaphore that the end-of-program barrier waits on per
            # engine queue at ~115ns each — merging tiles shrinks the
            # ~12us teardown tail.
            qt = resp.tile([128, IT * N], F16, tag="qt", name="qt")
            # Per-head zero-padded K^T: rows r:r+64 hold head h's K, the
            # other 64 rows are zero.  This makes every matmul in the
            # attention stream a 128-row-stationary op — PE tile-geometry
            # (row-size) switches between 64- and 128-row stationaries cost
            # ~105ns each, ~250 times per kernel.
            kz = resp.tile([128, HG * M], F16, tag="kz", name="kz")
            va = resp.tile([128, MT * HG * 65], F16, tag="va", name="va")
            # aot split by n-half so half1 oproj never (falsely) waits on
            # half1 normalize writes.
            aoh = [resp.tile([128, IT * 1024], F16, tag=f"ao{j}", name=f"ao{j}")
                   for j in range(2)]
            wo_sb = resp.tile([128, IT, QD], F16, tag="wo", name="wo")
            wq_sb = qpp.tile([128, KQ, IS], F16, tag="wq", name="wq")
            xh = qpp.tile([128, KQ, N], F16, tag="xh", name="xh")
            ch = kvp.tile([128, KC, M], F16, tag="ch", name="ch")
            wk_sb = kvp.tile([128, KC, IS], F16, tag="wk", name="wk")
            wv_sb = kvp.tile([128, KC, IS], F16, tag="wv", name="wv")

            # Zero the pad halves of kz on GpSimd (idle during input DMA).
            for h in range(HG):
                r = (h % 2) * 64
                nc.gpsimd.memset(kz[64 - r:128 - r, h * M:(h + 1) * M], 0.0)

            def blk(src, k0, k1):
                return src[k0 * 128:k1 * 128, :].rearrange(
                    "(k p) f -> p k f", p=128)

            # Input DMAs on two hardware queues (aggregate HBM bandwidth is
            # the binding constraint at ~25us for 8.9MB; the split just
            # lets kvproj start early while xh streams).  Sync: kvproj's
            # tensors per-k + wv; Scalar queue: qproj/oproj tensors as a
            # few large strided transfers.
            for k in range(KC):
                ksl = slice(k * 128, (k + 1) * 128)
                nc.sync.dma_start(wk_sb[:, k, :], wk[ksl, :])
                nc.sync.dma_start(ch[:, k, :], cT[ksl, :])
            nc.scalar.dma_start(wq_sb[:], blk(wq, 0, 8))
            nc.scalar.dma_start(xh[:, 0:4, :], blk(xT, 0, 4))
            nc.scalar.dma_start(wv_sb[:], blk(wv, 0, 6))
            nc.scalar.dma_start(xh[:, 4:8, :], blk(xT, 4, 8))
            nc.scalar.dma_start(wo_sb[:], blk(wo, 0, 4))

            def ps2():
                return psp.tile([128, 1024], F32, tag="mm", name="mm")

            # ------------- K^T -> kz (per-head, zero-padded) -------------
            with nc.named_scope("kvproj"):
                for i in range(IT):
                    isl = slice(i * 128, (i + 1) * 128)
                    pk = ps2()
                    for k in range(KC):
                        for jm in range(2):
                            nc.tensor.matmul(pk[:, jm * 512:(jm + 1) * 512],
                                             wk_sb[:, k, isl],
                                             ch[:, k, jm * 512:(jm + 1) * 512],
                                             start=(k == 0), stop=(k == KC - 1))
                    nc.vector.tensor_copy(
                        kz[0:64, (2 * i) * M:(2 * i + 1) * M], pk[0:64, :])
                    nc.vector.tensor_copy(
                        kz[64:128, (2 * i + 1) * M:(2 * i + 2) * M],
                        pk[64:128, :])

            def va_fill(mi, src):
                hcol = va[:, mi * 520:(mi + 1) * 520].rearrange(
                    "p (h c) -> p h c", c=65)
                psv = src.rearrange("p (h c) -> p h c", c=64)
                nc.vector.memset(hcol[:, :, 64], 1.0)
                nc.vector.tensor_copy(hcol[:, :, 0:64], psv[:])

            def vproj_chunk(mi):
                # V -> va for one m-chunk, on the 1-bank dribble pool
                pd = drp.tile([128, 512], F32, tag="dr", name="dr")
                msl = slice(mi * 128, (mi + 1) * 128)
                for k in range(KC):
                    nc.tensor.matmul(pd[:], ch[:, k, msl], wv_sb[:, k, :],
                                     start=(k == 0), stop=(k == KC - 1))
                va_fill(mi, pd[:])

            # First half of vproj runs pre-attention: it needs only ch+wv,
            # so it fills PE idle while the 4MB xh transfer streams in.
            with nc.named_scope("vproj"):
                for mp in range(2):
                    pt = ps2()
                    for k in range(KC):
                        for jm in range(2):
                            mi = 2 * mp + jm
                            msl = slice(mi * 128, (mi + 1) * 128)
                            nc.tensor.matmul(pt[:, jm * 512:(jm + 1) * 512],
                                             ch[:, k, msl], wv_sb[:, k, :],
                                             start=(k == 0), stop=(k == KC - 1))
                    for jm in range(2):
                        mi = 2 * mp + jm
                        va_fill(mi, pt[:, jm * 512:(jm + 1) * 512])

            # ------------- Q^T -> qt -------------
            def qproj_mm(i, jn, ps, psl):
                isl = slice(i * 128, (i + 1) * 128)
                for k in range(KQ):
                    nc.tensor.matmul(ps[:, psl], wq_sb[:, k, isl],
                                     xh[:, k, jn * 512:(jn + 1) * 512],
                                     start=(k == 0), stop=(k == KQ - 1))

            # Only head-pair 0's q is projected up front (it gates the
            # first sim); the rest dribbles into the attention windows.
            with nc.named_scope("qproj0"):
                for i in range(1):
                    pq = ps2()
                    for jn in range(2):
                        qproj_mm(i, jn, pq, slice(jn * 512, (jn + 1) * 512))
                    nc.vector.tensor_copy(qt[:, i * N:i * N + 1024], pq[:])

            # dribble generators: emitted one chunk per attention unit to
            # fill PE while ACT works through exp.  Order matters (PE
            # executes in emission order): qt[2]/qt[3] first-half chunks
            # must land before sim of heads 4..7 is emitted.
            def qdrib(chunks):
                for i, jn in chunks:
                    pd = drp.tile([128, 512], F32, tag="dr", name="dr")
                    qproj_mm(i, jn, pd, slice(0, 512))
                    nc.vector.tensor_copy(
                        qt[:, i * N + jn * 512:i * N + (jn + 1) * 512], pd[:])
                    yield

            def oproj_half(nts):
                for nt in nts:
                    tsl = slice((nt % 8) * 128, (nt % 8 + 1) * 128)
                    ao = aoh[nt // 8]
                    ob = obp.tile([128, QD], F16, tag="ob", name="ob")
                    for half in range(2):
                        qsl = slice(half * 512, (half + 1) * 512)
                        pd = drp.tile([128, 512], F32, tag="dr", name="dr")
                        for ik in range(IT):
                            nc.tensor.matmul(
                                pd[:],
                                ao[:, ik * 1024 + tsl.start:ik * 1024 + tsl.stop],
                                wo_sb[:, ik, qsl],
                                start=(ik == 0), stop=(ik == IT - 1))
                        nc.vector.tensor_copy(ob[:, qsl], pd[:])
                        yield
                    nc.sync.dma_start(out[nt * 128:(nt + 1) * 128, :], ob[:])

            def chain(*gens):
                for g in gens:
                    yield from g

            # ---------------- attention ----------------
            with nc.named_scope("attn"):
                for jnp in range(2):
                    if jnp == 0:
                        dribble = qdrib([(1, 0), (1, 1), (2, 0), (2, 1),
                                         (3, 0), (3, 1), (0, 2), (1, 2),
                                         (2, 2), (0, 3), (1, 3)])
                    else:
                        dribble = chain(qdrib([(2, 3), (3, 2), (3, 3)]),
                                        oproj_half(range(8)))
                    es_cur = {}

                    def sim_block(h, mi, jnp=jnp, es_cur=None):
                        i = h // 2
                        ktsl = kz[:, h * M + mi * 128:h * M + (mi + 1) * 128]
                        pss = ps2()
                        for sub in range(2):
                            jn = jnp * 2 + sub
                            nc.tensor.matmul(
                                pss[:, sub * 512:(sub + 1) * 512], ktsl,
                                qt[:, i * N + jn * 512:i * N + (jn + 1) * 512],
                                start=True, stop=True)
                        es = esp.tile([128, 1024], F16, tag=f"es{mi}",
                                      name=f"es{mi}", bufs=2)
                        nc.scalar.activation(es[:], pss[:], EXP)
                        es_cur[mi] = es

                    for mi in range(MT):
                        sim_block(0, mi, es_cur=es_cur)
                        if jnp == 0:
                            # va[mi 4..7] must be emitted before attn@v of
                            # head 0 (mi 0..3 were projected pre-attention)
                            if mi >= 4:
                                vproj_chunk(mi)
                            # qt[1] chunks must land before sim of heads 2/3
                            if mi % 2 == 1 and (mi < 4 or mi == 7):
                                next(dribble, None)
                        elif mi % 2 == 1:
                            next(dribble, None)
                    for h in range(HG):
                        po = [pop.tile([65, 512], F32, tag="po", name="po")
                              for _ in range(2)]
                        es_prev, es_cur = es_cur, {}
                        for mi in range(MT):
                            for sub in range(2):
                                nc.tensor.matmul(
                                    po[sub][:],
                                    va[:, mi * 520 + h * 65:
                                         mi * 520 + h * 65 + 65],
                                    es_prev[mi][:, sub * 512:(sub + 1) * 512],
                                    start=(mi == 0), stop=(mi == MT - 1))
                            if h + 1 < HG:
                                sim_block(h + 1, mi, es_cur=es_cur)
                            elif mi % 2 == 0:
                                next(dribble, None)
                            if mi % 4 == 3:
                                next(dribble, None)
                        i, r = h // 2, (h % 2) * 64
                        for sub in range(2):
                            # denominator row -> partition 0 first: the
                            # approx reciprocal (custom DVE op) requires
                            # SBUF input at partition offset 0.
                            drow = nmp.tile([1, 512], F32, tag="drow",
                                            name="drow", bufs=2)
                            nc.vector.tensor_copy(drow[:], po[sub][64:65, :])
                            rf = nmp.tile([1, 512], F32, tag="rf", name="rf",
                                          bufs=2)
                            nc.vector.reciprocal_approx_fast(rf[:], drow[:])
                            pbs = nmp.tile([64, 512], F32, tag="pbs",
                                           name="pbs", bufs=2)
                            nc.gpsimd.partition_broadcast(pbs[:], rf[:])
                            nsl = slice(i * 1024 + sub * 512,
                                        i * 1024 + (sub + 1) * 512)
                            nc.vector.tensor_mul(aoh[jnp][r:r + 64, nsl],
                                                 po[sub][0:64, :], pbs[:])
                    # drain any dribbles not consumed during this half
                    for _ in dribble:
                        pass

            # -------- out tail: second n-half of oproj --------
            # uses the (now free) 2-buf mm pool for pipelining; ob copies
            # go through the Scalar engine, idle once attention is done.
            with nc.named_scope("oproj"):
                for nt in range(8, NT):
                    t0 = (nt - 8) * 128
                    pso = ps2()
                    for ik in range(IT):
                        for half in range(2):
                            qsl = slice(half * 512, (half + 1) * 512)
                            nc.tensor.matmul(
                                pso[:, qsl],
                                aoh[1][:, ik * 1024 + t0:ik * 1024 + t0 + 128],
                                wo_sb[:, ik, qsl],
                                start=(ik == 0), stop=(ik == IT - 1))
                    ob = obp.tile([128, QD], F16, tag="ob", name="ob")
                    # alternate the PSUM->SBUF copies between Scalar and
                    # Vector so neither engine paces the 2-buf pso rotation;
                    # the last tile goes out per-half so the final DMA and
                    # its completion wait are small.
                    if nt == NT - 1:
                        nc.scalar.copy(ob[:, 0:512], pso[:, 0:512])
                        nc.sync.dma_start(out[nt * 128:(nt + 1) * 128, 0:512],
                                          ob[:, 0:512])
                        nc.vector.tensor_copy(ob[:, 512:1024], pso[:, 512:1024])
                        nc.sync.dma_start(out[nt * 128:(nt + 1) * 128, 512:1024],
                                          ob[:, 512:1024])
                        continue
                    if nt % 2 == 0:
                        nc.scalar.copy(ob[:], pso[:])
                    else:
                        nc.vector.tensor_copy(ob[:], pso[:])
                    nc.sync.dma_start(out[nt * 128:(nt + 1) * 128, :], ob[:])
    nc.compile()
    return nc


_NC_CACHE = None


def kernel(x, context, Wq, Wk, Wv, Wo, bo, _trace=False):
    global _NC_CACHE, LAST_RESULTS
    f16 = np.float16

    x = np.asarray(x, np.float32)
    context = np.asarray(context, np.float32)
    Wq = np.asarray(Wq, np.float32)
    Wk = np.asarray(Wk, np.float32)
    Wv = np.asarray(Wv, np.float32)
    Wo = np.asarray(Wo, np.float32)
    scale = np.float32(DH ** -0.5)

    if _NC_CACHE is None:
        _NC_CACHE = build_nc()
    nc = _NC_CACHE

    in_maps = []
    for c in range(NC):
        b, g = c // 2, c % 2
        sl = slice(g * IS, (g + 1) * IS)
        in_maps.append({
            "xT": np.ascontiguousarray(x[b].T).astype(f16),
            "cT": np.ascontiguousarray(context[b].T).astype(f16),
            "wq": (Wq[:, sl] * scale).astype(f16),
            "wk": np.ascontiguousarray(Wk[:, sl]).astype(f16),
            "wv": np.ascontiguousarray(Wv[:, sl]).astype(f16),
            "wo": np.ascontiguousarray(Wo[sl, :]).astype(f16),
        })
    res = run_bass_kernel_spmd(nc, in_maps, core_ids=list(range(NC)),
                               trace=_trace)
    LAST_RESULTS = res
    out = np.empty((B, N, QD), np.float32)
    bo32 = np.asarray(bo, np.float32)
    for b in range(B):
        out[b] = (res.results[2 * b]["out"].astype(np.float32)
                  + res.results[2 * b + 1]["out"].astype(np.float32) + bo32)
    return out
